# revision 10
# baseline (speedup 1.0000x reference)
"""nn_BinaryQuadratic Trainium2 kernel (8 NeuronCores, SPMD) — fp8 DoubleRow.

Math (per reference):
    Yb = (Y > 0.5), Zb = (Z > 0.5)                      # binary codebooks
    W[bit,rw,cw] = a*Yb@Zb + b*Ysum + c*Zsum            # [512, 512] blocks
    W = sum_bit W + d  -> permute -> [4096, 4096]
    out = X @ W.T + bias

Sharding: tensor-parallel over rw (8 row blocks of W <-> 8 output column
blocks of out). Core i builds a [512, 4096] weight slice for rw=i on
device and computes out.T = W_slice @ X.T -> [512, 4096]. Host
transposes/concatenates the 8 slices.

Precision split. With Ys = sign(Y-0.5), Zs = sign(Z-0.5):
    W^T[k,y] = Wg[k,y] + svec[k]
    Wg[k,y]  = sum_i lhs[i,k]*Ys[i,y],  lhs = (a/4)Zs + (a/4 + b/2)
    svec[k]  = sum_b (a/4 + c/2)*colsum(Zs)[k] + dpp
Wg has entry std ~10 while svec (via the dpp constant) has std ~96 and
dominates the output.  The device computes only X @ Wg.T, in fp8e4
(DoubleRow, 2 MACs/cell/cycle); the dominant rank-1 svec term and bias
are folded on the host into ubb[m,y] = (X @ svec)[m] + bias[y], which
the DVE adds exactly (f32) during PSUM evacuation.  Total rms error
~4e-3 vs the f32 reference (budget 2e-2).

Device pipeline per core:
  Phase A (codebook): per cw, DMA fp8 lhs/Ys pair-tiles; one DoubleRow
    matmul per 128-k chunk (contraction 256 = 4 bits x 64 inter) builds
    Wg^T [128, 512] in PSUM; DVE/ACT alternate evacuating to fp8 wt_sb.
  Phase B (main GEMM, transposed output): per m-group (512 cols of X^T),
    4 PSUM banks (one per 128-y chunk) accumulate 16 DoubleRow matmuls
    (stationary = wt_sb [128, 2, 128], moving = X^T fp8 [128, 2, 512]).
    Banks double-buffer across m-groups; DVE evacuates with the exact
    f32 ubb add; GpSimd DMAs each [128, 512] f32 block out.

PE warm-up matmuls run during the DMA lead-in (the PE drops to a low
p-state after idling and takes ~3us to reach full clock).
"""

import numpy as np
import ml_dtypes

import concourse.mybir as mybir
import concourse.tile as tile
from concourse import bacc
from concourse.bass_utils import run_bass_kernel_spmd

BIT, RW, CW, YR, ID, ZC = 4, 8, 8, 512, 64, 512
P = 128
NPAIR = 2   # bit pairs side by side in the free dim (DoubleRow j)
KT = 32     # 4096 / 128 contraction tiles
MG = 8      # m-groups of 512 columns of X^T
YC = 4      # 128-row y chunks of the per-core 512-row W slice
F32 = mybir.dt.float32
FP8 = mybir.dt.float8e4
BF16 = mybir.dt.bfloat16
FP8NP = ml_dtypes.float8_e4m3
DR = mybir.MatmulPerfMode.DoubleRow

_CACHE = {}


def _patch_compiler():
    """Disable the in-compile BIR simulator (compile-time only). Idempotent."""
    import concourse.bass_utils as bu

    if getattr(bu, "_bq_patched", False):
        return
    orig = bu.bir_verify_and_optimise

    def patched(tmpdir, inp="bir.json", outp="file.neff", arch=None, *, dve_root=None):
        real_run = bu.run_command

        def run(argv, **kw):
            argv = list(argv)
            for i, arg in enumerate(argv):
                if arg == "--enable-birsim=true":
                    argv[i] = "--enable-birsim=false"
            return real_run(argv, **kw)

        bu.run_command = run
        try:
            return orig(tmpdir, inp, outp, arch, dve_root=dve_root)
        finally:
            bu.run_command = real_run

    bu.bir_verify_and_optimise = patched
    bu._bq_patched = True


def _build_nc():
    nc = bacc.Bacc("TRN2", target_bir_lowering=False, debug=False)

    # X^T, fp8: xb[mg, p, kt, m] = X[mg*512+m, kt*128+p]
    xb = nc.dram_tensor("xb", [MG, P, KT, 512], FP8, kind="ExternalInput").ap()
    # lhs/Ys pair-tiles, fp8: [pair, cw, p=2*64, {z|y}]
    lhsp = nc.dram_tensor("lhsp", [NPAIR, CW, P, ZC], FP8, kind="ExternalInput").ap()
    ybp = nc.dram_tensor("ybp", [NPAIR, CW, P, YR], FP8, kind="ExternalInput").ap()
    # exact rank-1 + bias correction: ubb[mg, yc, p, m] = u[mg*512+m] + bias[yc*128+p]
    ubb = nc.dram_tensor("ubb", [MG, YC, P, 512], F32, kind="ExternalInput").ap()
    # transposed output blocks: outT[mg, yc, p, m]
    outT = nc.dram_tensor("outT", [MG, YC, P, 512], BF16, kind="ExternalOutput").ap()

    def kern(tc: tile.TileContext):
        nc = tc.nc
        from contextlib import ExitStack

        with ExitStack() as ctx:
            const = ctx.enter_context(tc.tile_pool(name="const", bufs=1))
            wtpool = ctx.enter_context(tc.tile_pool(name="wt", bufs=1))
            xpool = ctx.enter_context(tc.tile_pool(name="xg", bufs=3))
            upool = ctx.enter_context(tc.tile_pool(name="ub", bufs=2))
            apool = ctx.enter_context(tc.tile_pool(name="phA", bufs=8))
            opool = ctx.enter_context(tc.tile_pool(name="osb", bufs=4))
            psp = ctx.enter_context(tc.tile_pool(name="ps", bufs=8, space="PSUM"))

            # PE warm-up on zeroed SBUF during the DMA lead-in
            warm = const.tile([P, YR], FP8)
            nc.vector.memset(warm[:], 0.0)
            warm_ps = psp.tile([P, YR], F32, tag="ps", name="warm_ps")
            for _ in range(8):
                nc.tensor.matmul(warm_ps[:], warm[:, 0:P], warm[:], start=True, stop=True)

            # Wg^T slice, fp8: [z_in, kt=cw*4+zt, y]
            wt_sb = wtpool.tile([P, KT, YR], FP8)

            # X^T m-group tiles; mg0/mg1 DMAs issued during phase A below
            xgs = []

            def xg_dma(mg):
                xgs.append(xpool.tile([P, KT, 512], FP8, tag="xg", name=f"xg{mg}"))
                nc.sync.dma_start(xgs[mg][:], xb[mg])

            # ---- Phase A: build Wg^T ----
            # all codebook DMAs first (small, 2MB total), then the big X
            # m-group streams behind them on the sync ring
            ab = []
            for cw in range(CW):
                lhs2 = apool.tile([P, NPAIR, ZC], FP8, tag="lhs2", name=f"lhs2_{cw}")
                nc.sync.dma_start(lhs2[:], lhsp[:, cw].rearrange("n p z -> p n z"))
                yb2 = apool.tile([P, NPAIR, YR], FP8, tag="yb2", name=f"yb2_{cw}")
                nc.sync.dma_start(yb2[:], ybp[:, cw].rearrange("n p y -> p n y"))
                ab.append((lhs2, yb2))
            xg_dma(0)
            xg_dma(1)
            for cw in range(CW):
                lhs2, yb2 = ab[cw]
                for zt in range(4):
                    kt = cw * 4 + zt
                    zsl = slice(zt * P, (zt + 1) * P)
                    w_ps = psp.tile([P, YR], F32, tag="ps")
                    nc.tensor.matmul(
                        w_ps[:],
                        lhs2[:, :, zsl],
                        yb2[:, :, :],
                        start=True,
                        stop=True,
                        perf_mode=DR,
                    )
                    # evacuate to fp8; alternate DVE/ACT so neither gates PE
                    if kt % 2 == 0:
                        nc.vector.tensor_copy(wt_sb[:, kt, :], w_ps[:])
                    else:
                        nc.scalar.activation(
                            wt_sb[:, kt, :],
                            w_ps[:],
                            mybir.ActivationFunctionType.Identity,
                        )

            # ---- Phase B: out.T = Wg @ X.T + ubb, PSUM-accumulated over k ----
            for mg in range(MG):
                if mg + 2 < MG:
                    xg_dma(mg + 2)
                xg = xgs[mg]
                # ub rides the gpsimd ring: a buffer-gated dma_start here would
                # stall the sync ring and cascade into the B matmul stream
                ub4 = upool.tile([P, YC, 512], F32, tag="ub4", name=f"ub{mg}")
                nc.gpsimd.dma_start(ub4[:], ubb[mg].rearrange("c p m -> p c m"))
                ps = [
                    psp.tile([P, 512], F32, name=f"ps{mg}_{yc}", tag="ps")
                    for yc in range(YC)
                ]
                for dk in range(KT // 2):
                    for yc in range(YC):
                        nc.tensor.matmul(
                            ps[yc][:],
                            wt_sb[:, 2 * dk : 2 * dk + 2, yc * P : (yc + 1) * P],
                            xg[:, 2 * dk : 2 * dk + 2, :],
                            start=(dk == 0),
                            stop=(dk == KT // 2 - 1),
                            perf_mode=DR,
                        )
                for yc in range(YC):
                    osb = opool.tile([P, 512], BF16, tag="osb")
                    nc.vector.tensor_tensor(
                        osb[:], ps[yc][:], ub4[:, yc, :], mybir.AluOpType.add
                    )
                    nc.gpsimd.dma_start(outT[mg, yc], osb[:])

    with tile.TileContext(nc) as tc:
        kern(tc)
    nc.compile()
    return nc


def _prep_inputs(X, Y, Z, a, b, c, d, bias):
    """Host-side layout transforms + scalar folding + rank-1 term."""
    X = np.asarray(X, dtype=np.float32)
    # xb[mg, p, kt, m] = X[mg*512+m, kt*128+p], fp8
    XT = np.ascontiguousarray(X.T)  # [k, m]
    xb = np.ascontiguousarray(
        XT.reshape(KT, P, MG, 512).transpose(2, 1, 0, 3).astype(FP8NP)
    )
    Y = np.asarray(Y, dtype=np.float32)
    Z = np.asarray(Z, dtype=np.float32)
    a = np.asarray(a, dtype=np.float32).reshape(BIT, RW, CW)
    b = np.asarray(b, dtype=np.float32).reshape(BIT, RW, CW)
    c = np.asarray(c, dtype=np.float32).reshape(BIT, RW, CW)
    d = np.asarray(d, dtype=np.float32).reshape(RW, CW)
    bias = np.asarray(bias, dtype=np.float32)

    # +/-1 codebooks: Yb=(Ys+1)/2, Zb=(Zs+1)/2 expansion
    Ys = np.where(Y > 0.5, np.float32(1.0), np.float32(-1.0))
    Zs = np.where(Z > 0.5, np.float32(1.0), np.float32(-1.0))
    a4 = a / 4.0
    beta = a / 4.0 + b / 2.0
    gamma = a / 4.0 + c / 2.0
    dpp = d + (16.0 * a + 32.0 * b + 32.0 * c).sum(axis=0)  # [RW, CW]
    # svec[rw, cw, z] = sum_bit gamma * colsum(Zs) + dpp  (rank-1 in y)
    zcol = Zs.sum(axis=3)  # [bit, rw, cw, z]
    svec = np.einsum("brc,brcz->rcz", gamma, zcol) + dpp[:, :, None]
    # u[rw, m] = X @ svec[rw]  (exact f32 on host)
    u = X @ svec.reshape(RW, CW * ZC).T  # [4096 m, RW]

    in_maps = []
    for rw in range(RW):
        # Y[bit, rw, cw, y, i] -> ybp[pair, cw, j*64+i, y], bit = 2*pair + j
        Yt = Ys[:, rw].transpose(0, 1, 3, 2)  # [bit, cw, i, y]
        YP = np.ascontiguousarray(
            Yt.reshape(NPAIR, 2, CW, ID, YR).transpose(0, 2, 1, 3, 4).astype(FP8NP)
        ).reshape(NPAIR, CW, P, YR)
        # lhs[bit, rw, cw, i, z] = a4*Zs + beta -> same pair packing
        lhs = a4[:, rw, :, None, None] * Zs[:, rw] + beta[:, rw, :, None, None]
        LP = np.ascontiguousarray(
            lhs.reshape(NPAIR, 2, CW, ID, ZC).transpose(0, 2, 1, 3, 4).astype(FP8NP)
        ).reshape(NPAIR, CW, P, ZC)
        # ubb[mg, yc, p, m] = u[mg*512+m] + bias[yc*128+p]
        ub = (
            u[:, rw].reshape(MG, 1, 1, 512)
            + bias[rw * YR : (rw + 1) * YR].reshape(1, YC, P, 1)
        ).astype(np.float32)
        in_maps.append({"xb": xb, "lhsp": LP, "ybp": YP, "ubb": np.ascontiguousarray(ub)})
    return in_maps


def _get_nc():
    if "nc" not in _CACHE:
        _patch_compiler()
        _CACHE["nc"] = _build_nc()
    return _CACHE["nc"]


def kernel(X, Y, Z, a, b, c, d, bias, _trace=False):
    nc = _get_nc()
    in_maps = _prep_inputs(X, Y, Z, a, b, c, d, bias)
    try:
        res = run_bass_kernel_spmd(nc, in_maps, core_ids=list(range(RW)), trace=_trace)
    except Exception:
        # transient NRT_EXEC_UNIT_UNRECOVERABLE flakes have been observed
        # on first device touch; one retry clears them
        res = run_bass_kernel_spmd(nc, in_maps, core_ids=list(range(RW)), trace=_trace)
    parts = []
    for rw in range(RW):
        oT = np.asarray(res.results[rw]["outT"], dtype=np.float32)  # [MG, YC, P, 512]
        parts.append(
            np.ascontiguousarray(oT.transpose(0, 3, 1, 2)).reshape(MG * 512, YC * P)
        )
    full = np.concatenate(parts, axis=1)
    if _trace:
        _CACHE["last_result"] = res
    return full


# revision 13
# speedup vs baseline: 1.0439x; 1.0439x over previous
"""nn_BinaryQuadratic Trainium2 kernel (8 NeuronCores, SPMD) — fp8 DoubleRow.

Math (per reference):
    Yb = (Y > 0.5), Zb = (Z > 0.5)                      # binary codebooks
    W[bit,rw,cw] = a*Yb@Zb + b*Ysum + c*Zsum            # [512, 512] blocks
    W = sum_bit W + d  -> permute -> [4096, 4096]
    out = X @ W.T + bias

Sharding: tensor-parallel over rw (8 row blocks of W <-> 8 output column
blocks of out). Core i builds a [512, 4096] weight slice for rw=i on
device and computes out.T = W_slice @ X.T -> [512, 4096]. Host
transposes/concatenates the 8 slices.

Precision split. With Ys = sign(Y-0.5), Zs = sign(Z-0.5):
    W^T[k,y] = Wg[k,y] + svec[k]
    Wg[k,y]  = sum_i lhs[i,k]*Ys[i,y],  lhs = (a/4)Zs + (a/4 + b/2)
    svec[k]  = sum_b (a/4 + c/2)*colsum(Zs)[k] + dpp
Wg has entry std ~10 while svec (via the dpp constant) has std ~96 and
dominates the output.  The device computes only X @ Wg.T, in fp8e4
(DoubleRow, 2 MACs/cell/cycle); the dominant rank-1 svec term and bias
are folded on the host into ubb[m,y] = (X @ svec)[m] + bias[y], which
the DVE adds exactly (f32) during PSUM evacuation.  Total rms error
~4e-3 vs the f32 reference (budget 2e-2).

Device pipeline per core:
  Phase A (codebook): per cw, DMA fp8 lhs/Ys pair-tiles; one DoubleRow
    matmul per 128-k chunk (contraction 256 = 4 bits x 64 inter) builds
    Wg^T [128, 512] in PSUM; DVE/ACT alternate evacuating to fp8 wt_sb.
  Phase B (main GEMM, transposed output): per m-group (512 cols of X^T),
    4 PSUM banks (one per 128-y chunk) accumulate 16 DoubleRow matmuls
    (stationary = wt_sb [128, 2, 128], moving = X^T fp8 [128, 2, 512]).
    Banks double-buffer across m-groups; DVE evacuates with the exact
    f32 ubb add; GpSimd DMAs each [128, 512] f32 block out.

PE warm-up matmuls run during the DMA lead-in (the PE drops to a low
p-state after idling and takes ~3us to reach full clock).
"""

import numpy as np
import ml_dtypes

import concourse.mybir as mybir
import concourse.tile as tile
from concourse import bacc
from concourse.bass_utils import run_bass_kernel_spmd

BIT, RW, CW, YR, ID, ZC = 4, 8, 8, 512, 64, 512
P = 128
NPAIR = 2   # bit pairs side by side in the free dim (DoubleRow j)
KT = 32     # 4096 / 128 contraction tiles
MG = 8      # m-groups of 512 columns of X^T
YC = 4      # 128-row y chunks of the per-core 512-row W slice
F32 = mybir.dt.float32
FP8 = mybir.dt.float8e4
BF16 = mybir.dt.bfloat16
FP8NP = ml_dtypes.float8_e4m3
DR = mybir.MatmulPerfMode.DoubleRow

_CACHE = {}


def _patch_compiler():
    """Disable the in-compile BIR simulator (compile-time only). Idempotent."""
    import concourse.bass_utils as bu

    if getattr(bu, "_bq_patched", False):
        return
    orig = bu.bir_verify_and_optimise

    def patched(tmpdir, inp="bir.json", outp="file.neff", arch=None, *, dve_root=None):
        real_run = bu.run_command

        def run(argv, **kw):
            argv = list(argv)
            for i, arg in enumerate(argv):
                if arg == "--enable-birsim=true":
                    argv[i] = "--enable-birsim=false"
            return real_run(argv, **kw)

        bu.run_command = run
        try:
            return orig(tmpdir, inp, outp, arch, dve_root=dve_root)
        finally:
            bu.run_command = real_run

    bu.bir_verify_and_optimise = patched
    bu._bq_patched = True


def _build_nc():
    nc = bacc.Bacc("TRN2", target_bir_lowering=False, debug=False)

    # X^T, fp8: xb[mg, p, kt, m] = X[mg*512+m, kt*128+p]
    xb = nc.dram_tensor("xb", [MG, P, KT, 512], FP8, kind="ExternalInput").ap()
    # lhs/Ys pair-tiles, fp8: [pair, cw, p=2*64, {z|y}]
    lhsp = nc.dram_tensor("lhsp", [NPAIR, CW, P, ZC], FP8, kind="ExternalInput").ap()
    ybp = nc.dram_tensor("ybp", [NPAIR, CW, P, YR], FP8, kind="ExternalInput").ap()
    # exact rank-1 + bias correction: ubb[mg, yc, p, m] = u[mg*512+m] + bias[yc*128+p]
    ubb = nc.dram_tensor("ubb", [MG, YC, P, 512], F32, kind="ExternalInput").ap()
    # transposed output blocks: outT[mg, yc, p, m]
    outT = nc.dram_tensor("outT", [MG, YC, P, 512], BF16, kind="ExternalOutput").ap()

    def kern(tc: tile.TileContext):
        nc = tc.nc
        from contextlib import ExitStack

        with ExitStack() as ctx:
            const = ctx.enter_context(tc.tile_pool(name="const", bufs=1))
            wtpool = ctx.enter_context(tc.tile_pool(name="wt", bufs=1))
            xpool = ctx.enter_context(tc.tile_pool(name="xg", bufs=3))
            upool = ctx.enter_context(tc.tile_pool(name="ub", bufs=2))
            apool = ctx.enter_context(tc.tile_pool(name="phA", bufs=8))
            opool = ctx.enter_context(tc.tile_pool(name="osb", bufs=4))
            psp = ctx.enter_context(tc.tile_pool(name="ps", bufs=8, space="PSUM"))

            # PE warm-up on zeroed SBUF during the DMA lead-in
            warm = const.tile([P, YR], FP8)
            nc.vector.memset(warm[:], 0.0)
            warm_ps = psp.tile([P, YR], F32, tag="ps", name="warm_ps")
            for _ in range(10):
                nc.tensor.matmul(warm_ps[:], warm[:, 0:P], warm[:], start=True, stop=True)

            # Wg^T slice, fp8: [z_in, kt=cw*4+zt, y]
            wt_sb = wtpool.tile([P, KT, YR], FP8)

            # X^T m-group tiles; mg0/mg1 DMAs issued during phase A below
            xgs = []

            def xg_dma(mg):
                xgs.append(xpool.tile([P, KT, 512], FP8, tag="xg", name=f"xg{mg}"))
                nc.sync.dma_start(xgs[mg][:], xb[mg])

            # ---- Phase A: build Wg^T ----
            # all codebook DMAs first (small, 2MB total), then the big X
            # m-group streams behind them on the sync ring
            ab = []
            for cw in range(CW):
                lhs2 = apool.tile([P, NPAIR, ZC], FP8, tag="lhs2", name=f"lhs2_{cw}")
                nc.sync.dma_start(lhs2[:], lhsp[:, cw].rearrange("n p z -> p n z"))
                yb2 = apool.tile([P, NPAIR, YR], FP8, tag="yb2", name=f"yb2_{cw}")
                nc.sync.dma_start(yb2[:], ybp[:, cw].rearrange("n p y -> p n y"))
                ab.append((lhs2, yb2))
            # only xg0 before phase B: the sync ring publishes DMA-ready
            # semaphores in order (with an in-flight throttle), so extra
            # pre-issued m-groups would gate B's first matmul on THEIR
            # completion, not xg0's
            xg_dma(0)
            for cw in range(CW):
                lhs2, yb2 = ab[cw]
                for zt in range(4):
                    kt = cw * 4 + zt
                    zsl = slice(zt * P, (zt + 1) * P)
                    w_ps = psp.tile([P, YR], F32, tag="ps")
                    nc.tensor.matmul(
                        w_ps[:],
                        lhs2[:, :, zsl],
                        yb2[:, :, :],
                        start=True,
                        stop=True,
                        perf_mode=DR,
                    )
                    # evacuate to fp8; alternate DVE/ACT so neither gates PE
                    if kt % 2 == 0:
                        nc.vector.tensor_copy(wt_sb[:, kt, :], w_ps[:])
                    else:
                        nc.scalar.activation(
                            wt_sb[:, kt, :],
                            w_ps[:],
                            mybir.ActivationFunctionType.Identity,
                        )

            # ---- Phase B: out.T = Wg @ X.T + ubb, PSUM-accumulated over k ----
            for mg in range(MG):
                if mg + 1 < MG:
                    xg_dma(mg + 1)
                xg = xgs[mg]
                ub4 = upool.tile([P, YC, 512], F32, tag="ub4", name=f"ub{mg}")
                nc.sync.dma_start(ub4[:], ubb[mg].rearrange("c p m -> p c m"))
                ps = [
                    psp.tile([P, 512], F32, name=f"ps{mg}_{yc}", tag="ps")
                    for yc in range(YC)
                ]
                for dk in range(KT // 2):
                    for yc in range(YC):
                        nc.tensor.matmul(
                            ps[yc][:],
                            wt_sb[:, 2 * dk : 2 * dk + 2, yc * P : (yc + 1) * P],
                            xg[:, 2 * dk : 2 * dk + 2, :],
                            start=(dk == 0),
                            stop=(dk == KT // 2 - 1),
                            perf_mode=DR,
                        )
                for yc in range(YC):
                    osb = opool.tile([P, 512], BF16, tag="osb")
                    nc.vector.tensor_tensor(
                        osb[:], ps[yc][:], ub4[:, yc, :], mybir.AluOpType.add
                    )
                    nc.gpsimd.dma_start(outT[mg, yc], osb[:])

    with tile.TileContext(nc) as tc:
        kern(tc)
    nc.compile()
    return nc


def _prep_inputs(X, Y, Z, a, b, c, d, bias):
    """Host-side layout transforms + scalar folding + rank-1 term."""
    X = np.asarray(X, dtype=np.float32)
    # xb[mg, p, kt, m] = X[mg*512+m, kt*128+p], fp8
    XT = np.ascontiguousarray(X.T)  # [k, m]
    xb = np.ascontiguousarray(
        XT.reshape(KT, P, MG, 512).transpose(2, 1, 0, 3).astype(FP8NP)
    )
    Y = np.asarray(Y, dtype=np.float32)
    Z = np.asarray(Z, dtype=np.float32)
    a = np.asarray(a, dtype=np.float32).reshape(BIT, RW, CW)
    b = np.asarray(b, dtype=np.float32).reshape(BIT, RW, CW)
    c = np.asarray(c, dtype=np.float32).reshape(BIT, RW, CW)
    d = np.asarray(d, dtype=np.float32).reshape(RW, CW)
    bias = np.asarray(bias, dtype=np.float32)

    # +/-1 codebooks: Yb=(Ys+1)/2, Zb=(Zs+1)/2 expansion
    Ys = np.where(Y > 0.5, np.float32(1.0), np.float32(-1.0))
    Zs = np.where(Z > 0.5, np.float32(1.0), np.float32(-1.0))
    a4 = a / 4.0
    beta = a / 4.0 + b / 2.0
    gamma = a / 4.0 + c / 2.0
    dpp = d + (16.0 * a + 32.0 * b + 32.0 * c).sum(axis=0)  # [RW, CW]
    # svec[rw, cw, z] = sum_bit gamma * colsum(Zs) + dpp  (rank-1 in y)
    zcol = Zs.sum(axis=3)  # [bit, rw, cw, z]
    svec = np.einsum("brc,brcz->rcz", gamma, zcol) + dpp[:, :, None]
    # u[rw, m] = X @ svec[rw]  (exact f32 on host)
    u = X @ svec.reshape(RW, CW * ZC).T  # [4096 m, RW]

    in_maps = []
    for rw in range(RW):
        # Y[bit, rw, cw, y, i] -> ybp[pair, cw, j*64+i, y], bit = 2*pair + j
        Yt = Ys[:, rw].transpose(0, 1, 3, 2)  # [bit, cw, i, y]
        YP = np.ascontiguousarray(
            Yt.reshape(NPAIR, 2, CW, ID, YR).transpose(0, 2, 1, 3, 4).astype(FP8NP)
        ).reshape(NPAIR, CW, P, YR)
        # lhs[bit, rw, cw, i, z] = a4*Zs + beta -> same pair packing
        lhs = a4[:, rw, :, None, None] * Zs[:, rw] + beta[:, rw, :, None, None]
        LP = np.ascontiguousarray(
            lhs.reshape(NPAIR, 2, CW, ID, ZC).transpose(0, 2, 1, 3, 4).astype(FP8NP)
        ).reshape(NPAIR, CW, P, ZC)
        # ubb[mg, yc, p, m] = u[mg*512+m] + bias[yc*128+p]
        ub = (
            u[:, rw].reshape(MG, 1, 1, 512)
            + bias[rw * YR : (rw + 1) * YR].reshape(1, YC, P, 1)
        ).astype(np.float32)
        in_maps.append({"xb": xb, "lhsp": LP, "ybp": YP, "ubb": np.ascontiguousarray(ub)})
    return in_maps


def _get_nc():
    if "nc" not in _CACHE:
        _patch_compiler()
        _CACHE["nc"] = _build_nc()
    return _CACHE["nc"]


def kernel(X, Y, Z, a, b, c, d, bias, _trace=False):
    nc = _get_nc()
    in_maps = _prep_inputs(X, Y, Z, a, b, c, d, bias)
    try:
        res = run_bass_kernel_spmd(nc, in_maps, core_ids=list(range(RW)), trace=_trace)
    except Exception:
        # transient NRT_EXEC_UNIT_UNRECOVERABLE flakes have been observed
        # on first device touch; one retry clears them
        res = run_bass_kernel_spmd(nc, in_maps, core_ids=list(range(RW)), trace=_trace)
    parts = []
    for rw in range(RW):
        oT = np.asarray(res.results[rw]["outT"], dtype=np.float32)  # [MG, YC, P, 512]
        parts.append(
            np.ascontiguousarray(oT.transpose(0, 3, 1, 2)).reshape(MG * 512, YC * P)
        )
    full = np.concatenate(parts, axis=1)
    if _trace:
        _CACHE["last_result"] = res
    return full


# revision 15
# speedup vs baseline: 1.0761x; 1.0308x over previous
"""nn_BinaryQuadratic Trainium2 kernel (8 NeuronCores, SPMD) — fp8 DoubleRow.

Math (per reference):
    Yb = (Y > 0.5), Zb = (Z > 0.5)                      # binary codebooks
    W[bit,rw,cw] = a*Yb@Zb + b*Ysum + c*Zsum            # [512, 512] blocks
    W = sum_bit W + d  -> permute -> [4096, 4096]
    out = X @ W.T + bias

Sharding: tensor-parallel over rw (8 row blocks of W <-> 8 output column
blocks of out). Core i builds a [512, 4096] weight slice for rw=i on
device and computes out.T = W_slice @ X.T -> [512, 4096]. Host
transposes/concatenates the 8 slices.

Precision split. With Ys = sign(Y-0.5), Zs = sign(Z-0.5):
    W^T[k,y] = Wg[k,y] + svec[k]
    Wg[k,y]  = sum_i lhs[i,k]*Ys[i,y],  lhs = (a/4)Zs + (a/4 + b/2)
    svec[k]  = sum_b (a/4 + c/2)*colsum(Zs)[k] + dpp
Wg has entry std ~10 while svec (via the dpp constant) has std ~96 and
dominates the output.  The device computes only X @ Wg.T, in fp8e4
(DoubleRow, 2 MACs/cell/cycle); the dominant rank-1 svec term and bias
are folded on the host into ubb[m,y] = (X @ svec)[m] + bias[y], which
the DVE adds exactly (f32) during PSUM evacuation.  Total rms error
~4e-3 vs the f32 reference (budget 2e-2).

Device pipeline per core:
  Phase A (codebook): per cw, DMA fp8 lhs/Ys pair-tiles; one DoubleRow
    matmul per 128-k chunk (contraction 256 = 4 bits x 64 inter) builds
    Wg^T [128, 512] in PSUM; DVE/ACT alternate evacuating to fp8 wt_sb.
  Phase B (main GEMM, transposed output): per m-group (512 cols of X^T),
    4 PSUM banks (one per 128-y chunk) accumulate 16 DoubleRow matmuls
    (stationary = wt_sb [128, 2, 128], moving = X^T fp8 [128, 2, 512]).
    Banks double-buffer across m-groups; DVE evacuates with the exact
    f32 ubb add; GpSimd DMAs each [128, 512] f32 block out.

PE warm-up matmuls run during the DMA lead-in (the PE drops to a low
p-state after idling and takes ~3us to reach full clock).
"""

import numpy as np
import ml_dtypes

import concourse.mybir as mybir
import concourse.tile as tile
from concourse import bacc
from concourse.bass_utils import run_bass_kernel_spmd

BIT, RW, CW, YR, ID, ZC = 4, 8, 8, 512, 64, 512
P = 128
NPAIR = 2   # bit pairs side by side in the free dim (DoubleRow j)
KT = 32     # 4096 / 128 contraction tiles
MG = 8      # m-groups of 512 columns of X^T
YC = 4      # 128-row y chunks of the per-core 512-row W slice
F32 = mybir.dt.float32
FP8 = mybir.dt.float8e4
BF16 = mybir.dt.bfloat16
FP8NP = ml_dtypes.float8_e4m3
DR = mybir.MatmulPerfMode.DoubleRow

_CACHE = {}


def _patch_compiler():
    """Disable the in-compile BIR simulator (compile-time only). Idempotent."""
    import concourse.bass_utils as bu

    if getattr(bu, "_bq_patched", False):
        return
    orig = bu.bir_verify_and_optimise

    def patched(tmpdir, inp="bir.json", outp="file.neff", arch=None, *, dve_root=None):
        real_run = bu.run_command

        def run(argv, **kw):
            argv = list(argv)
            for i, arg in enumerate(argv):
                if arg == "--enable-birsim=true":
                    argv[i] = "--enable-birsim=false"
            return real_run(argv, **kw)

        bu.run_command = run
        try:
            return orig(tmpdir, inp, outp, arch, dve_root=dve_root)
        finally:
            bu.run_command = real_run

    bu.bir_verify_and_optimise = patched
    bu._bq_patched = True


def _build_nc():
    nc = bacc.Bacc("TRN2", target_bir_lowering=False, debug=False)

    # X^T, fp8: xb[mg, p, kt, m] = X[mg*512+m, kt*128+p]
    xb = nc.dram_tensor("xb", [MG, P, KT, 512], FP8, kind="ExternalInput").ap()
    # lhs/Ys codebook, fp8, packed for 8KB-contiguous partition lines:
    # lhsp[p, cw, pair, z], p = (bit%2)*64 + i, bit = 2*pair + p//64
    lhsp = nc.dram_tensor("lhsp", [P, CW, NPAIR, ZC], FP8, kind="ExternalInput").ap()
    ybp = nc.dram_tensor("ybp", [P, CW, NPAIR, YR], FP8, kind="ExternalInput").ap()
    # exact rank-1 + bias correction: ubb[mg, p, yc, m] = u[mg*512+m] + bias[yc*128+p]
    ubb = nc.dram_tensor("ubb", [MG, P, YC, 512], F32, kind="ExternalInput").ap()
    # transposed output blocks: outT[mg, yc, p, m]
    outT = nc.dram_tensor("outT", [MG, YC, P, 512], BF16, kind="ExternalOutput").ap()

    def kern(tc: tile.TileContext):
        nc = tc.nc
        from contextlib import ExitStack

        with ExitStack() as ctx:
            const = ctx.enter_context(tc.tile_pool(name="const", bufs=1))
            wtpool = ctx.enter_context(tc.tile_pool(name="wt", bufs=1))
            xpool = ctx.enter_context(tc.tile_pool(name="xg", bufs=3))
            upool = ctx.enter_context(tc.tile_pool(name="ub", bufs=2))
            apool = ctx.enter_context(tc.tile_pool(name="phA", bufs=1))
            opool = ctx.enter_context(tc.tile_pool(name="osb", bufs=4))
            psp = ctx.enter_context(tc.tile_pool(name="ps", bufs=8, space="PSUM"))

            # PE warm-up on zeroed SBUF during the DMA lead-in
            warm = const.tile([P, YR], FP8)
            nc.vector.memset(warm[:], 0.0)
            warm_ps = psp.tile([P, YR], F32, tag="ps", name="warm_ps")
            for _ in range(7):
                nc.tensor.matmul(warm_ps[:], warm[:, 0:P], warm[:], start=True, stop=True)

            # Wg^T slice, fp8: [z_in, kt=cw*4+zt, y]
            wt_sb = wtpool.tile([P, KT, YR], FP8)

            # X^T m-group tiles; mg0/mg1 DMAs issued during phase A below
            xgs = []

            def xg_dma(mg):
                xgs.append(xpool.tile([P, KT, 512], FP8, tag="xg", name=f"xg{mg}"))
                nc.sync.dma_start(xgs[mg][:], xb[mg])

            # ---- Phase A: build Wg^T ----
            # codebook as two single DMAs (8KB contiguous per partition ->
            # ~256 descriptors instead of ~6000 512B ones), ahead of the big
    # X m-group streams on the sync ring
            lhs_all = apool.tile([P, CW, NPAIR, ZC], FP8, name="lhs_all")
            nc.sync.dma_start(lhs_all[:], lhsp)
            yb_all = apool.tile([P, CW, NPAIR, YR], FP8, name="yb_all")
            nc.sync.dma_start(yb_all[:], ybp)
            # only xg0 before phase B: the sync ring publishes DMA-ready
            # semaphores in order (with an in-flight throttle), so extra
            # pre-issued m-groups would gate B's first matmul on THEIR
            # completion, not xg0's
            xg_dma(0)
            for cw in range(CW):
                lhs2 = lhs_all[:, cw]
                yb2 = yb_all[:, cw]
                for zt in range(4):
                    kt = cw * 4 + zt
                    zsl = slice(zt * P, (zt + 1) * P)
                    w_ps = psp.tile([P, YR], F32, tag="ps")
                    nc.tensor.matmul(
                        w_ps[:],
                        lhs2[:, :, zsl],
                        yb2[:, :, :],
                        start=True,
                        stop=True,
                        perf_mode=DR,
                    )
                    # evacuate to fp8; alternate DVE/ACT so neither gates PE
                    if kt % 2 == 0:
                        nc.vector.tensor_copy(wt_sb[:, kt, :], w_ps[:])
                    else:
                        nc.scalar.activation(
                            wt_sb[:, kt, :],
                            w_ps[:],
                            mybir.ActivationFunctionType.Identity,
                        )

            # ---- Phase B: out.T = Wg @ X.T + ubb, PSUM-accumulated over k ----
            for mg in range(MG):
                if mg + 1 < MG:
                    xg_dma(mg + 1)
                xg = xgs[mg]
                ub4 = upool.tile([P, YC, 512], F32, tag="ub4", name=f"ub{mg}")
                nc.sync.dma_start(ub4[:], ubb[mg])
                ps = [
                    psp.tile([P, 512], F32, name=f"ps{mg}_{yc}", tag="ps")
                    for yc in range(YC)
                ]
                for dk in range(KT // 2):
                    for yc in range(YC):
                        nc.tensor.matmul(
                            ps[yc][:],
                            wt_sb[:, 2 * dk : 2 * dk + 2, yc * P : (yc + 1) * P],
                            xg[:, 2 * dk : 2 * dk + 2, :],
                            start=(dk == 0),
                            stop=(dk == KT // 2 - 1),
                            perf_mode=DR,
                        )
                for yc in range(YC):
                    osb = opool.tile([P, 512], BF16, tag="osb")
                    nc.vector.tensor_tensor(
                        osb[:], ps[yc][:], ub4[:, yc, :], mybir.AluOpType.add
                    )
                    nc.gpsimd.dma_start(outT[mg, yc], osb[:])

    with tile.TileContext(nc) as tc:
        kern(tc)
    nc.compile()
    return nc


def _prep_inputs(X, Y, Z, a, b, c, d, bias):
    """Host-side layout transforms + scalar folding + rank-1 term."""
    X = np.asarray(X, dtype=np.float32)
    # xb[mg, p, kt, m] = X[mg*512+m, kt*128+p], fp8
    XT = np.ascontiguousarray(X.T)  # [k, m]
    xb = np.ascontiguousarray(
        XT.reshape(KT, P, MG, 512).transpose(2, 1, 0, 3).astype(FP8NP)
    )
    Y = np.asarray(Y, dtype=np.float32)
    Z = np.asarray(Z, dtype=np.float32)
    a = np.asarray(a, dtype=np.float32).reshape(BIT, RW, CW)
    b = np.asarray(b, dtype=np.float32).reshape(BIT, RW, CW)
    c = np.asarray(c, dtype=np.float32).reshape(BIT, RW, CW)
    d = np.asarray(d, dtype=np.float32).reshape(RW, CW)
    bias = np.asarray(bias, dtype=np.float32)

    # +/-1 codebooks: Yb=(Ys+1)/2, Zb=(Zs+1)/2 expansion
    Ys = np.where(Y > 0.5, np.float32(1.0), np.float32(-1.0))
    Zs = np.where(Z > 0.5, np.float32(1.0), np.float32(-1.0))
    a4 = a / 4.0
    beta = a / 4.0 + b / 2.0
    gamma = a / 4.0 + c / 2.0
    dpp = d + (16.0 * a + 32.0 * b + 32.0 * c).sum(axis=0)  # [RW, CW]
    # svec[rw, cw, z] = sum_bit gamma * colsum(Zs) + dpp  (rank-1 in y)
    zcol = Zs.sum(axis=3)  # [bit, rw, cw, z]
    svec = np.einsum("brc,brcz->rcz", gamma, zcol) + dpp[:, :, None]
    # u[rw, m] = X @ svec[rw]  (exact f32 on host)
    u = X @ svec.reshape(RW, CW * ZC).T  # [4096 m, RW]

    in_maps = []
    for rw in range(RW):
        # Y[bit, rw, cw, y, i] -> ybp[pair, cw, j*64+i, y], bit = 2*pair + j
        Yt = Ys[:, rw].transpose(0, 1, 3, 2)  # [bit, cw, i, y]
        # ybp[p, cw, pair, y], p = j*64+i, bit = 2*pair + j
        YP = np.ascontiguousarray(
            Yt.reshape(NPAIR, 2, CW, ID, YR).transpose(1, 3, 2, 0, 4).reshape(P, CW, NPAIR, YR).astype(FP8NP)
        )
        # lhs[bit, rw, cw, i, z] = a4*Zs + beta -> same packing
        lhs = a4[:, rw, :, None, None] * Zs[:, rw] + beta[:, rw, :, None, None]
        LP = np.ascontiguousarray(
            lhs.reshape(NPAIR, 2, CW, ID, ZC).transpose(1, 3, 2, 0, 4).reshape(P, CW, NPAIR, ZC).astype(FP8NP)
        )
        # ubb[mg, p, yc, m] = u[mg*512+m] + bias[yc*128+p]
        ub = (
            u[:, rw].reshape(MG, 1, 1, 512)
            + bias[rw * YR : (rw + 1) * YR].reshape(1, YC, P, 1).transpose(0, 2, 1, 3)
        ).astype(np.float32)
        in_maps.append({"xb": xb, "lhsp": LP, "ybp": YP, "ubb": np.ascontiguousarray(ub)})
    return in_maps


def _get_nc():
    if "nc" not in _CACHE:
        _patch_compiler()
        _CACHE["nc"] = _build_nc()
    return _CACHE["nc"]


def kernel(X, Y, Z, a, b, c, d, bias, _trace=False):
    nc = _get_nc()
    in_maps = _prep_inputs(X, Y, Z, a, b, c, d, bias)
    try:
        res = run_bass_kernel_spmd(nc, in_maps, core_ids=list(range(RW)), trace=_trace)
    except Exception:
        # transient NRT_EXEC_UNIT_UNRECOVERABLE flakes have been observed
        # on first device touch; one retry clears them
        res = run_bass_kernel_spmd(nc, in_maps, core_ids=list(range(RW)), trace=_trace)
    parts = []
    for rw in range(RW):
        oT = np.asarray(res.results[rw]["outT"], dtype=np.float32)  # [MG, YC, P, 512]
        parts.append(
            np.ascontiguousarray(oT.transpose(0, 3, 1, 2)).reshape(MG * 512, YC * P)
        )
    full = np.concatenate(parts, axis=1)
    if _trace:
        _CACHE["last_result"] = res
    return full
